# revision 4
# baseline (speedup 1.0000x reference)
"""Trainium2 Bass kernel for a Mamba block (nn_ATTD_MambaBlock).

Sharding: 2 (batch) x 4 (d_inner) grid over 8 NeuronCores.
Each core handles one batch element and a 384-channel slice of d_inner=1536.

Per-core pipeline (all layouts are [channels-on-partitions, seqlen-on-free]):
  1. in_proj x/z halves as fp16 PE matmuls (K=768 over 6 k-tiles).
  2. depthwise causal conv-4 as 4 accumulating diagonal-matrix PE matmuls.
  3. SiLU / softplus(=Ln(Exp+1)) / dA=exp(delta*A[:,n]) on the ACT engine
     (per-partition AP scale/bias).
  4. selective scan: DVE tensor_tensor_scan per (d-tile, n) plane, l on the
     free dim; chained across two l-halves via a carried last column.
  5. sum over n of C_n * h_n via accumulating identity matmuls into PSUM.
  6. gating (y + x*D) * silu(z), out_proj matmuls -> partial (768, L) fp32.
Host sums the 4 d-shard partials per batch (the "all-reduce") and transposes.
"""

import sys
import numpy as np

sys.path.insert(0, "/opt/trn_rl_repo")

import concourse.bass as bass  # noqa: E402
import concourse.tile as tile  # noqa: E402
from concourse import bacc, mybir  # noqa: E402
from contextlib import ExitStack  # noqa: E402

D_MODEL = 768
D_STATE = 16
D_CONV = 4
D_INNER = 1536
BATCH = 2
L = 2048
N_CORES = 8
D_SHARDS = 4
D_LOC = D_INNER // D_SHARDS      # 384
DT = D_LOC // 128                # 3 d-tiles of 128
KT = D_MODEL // 128              # 6 k-tiles for in_proj
MT = D_MODEL // 128              # 6 m-tiles for out_proj
NCH = 4                          # 512-column chunks per full l
H = L // 2                       # half length for the scan phase
HCH = H // 512                   # 512-column chunks per half

F16 = mybir.dt.float16
F32 = mybir.dt.float32
AF = mybir.ActivationFunctionType
OP = mybir.AluOpType

_PROG_CACHE = {}


def _build_program():
    nc = bacc.Bacc("TRN2", target_bir_lowering=False, debug=False,
                   num_devices=N_CORES)

    d = {}
    def di(name, shape, dtype):
        d[name] = nc.dram_tensor(name, list(shape), dtype, kind="ExternalInput").ap()

    di("hT", (128, KT, L), F16)            # hidden[b].T k-tiles: m = k*128+p
    di("w_in", (128, KT, 2 * D_LOC), F16)  # W_in shard^T k-tiles, x then z cols
    di("conv_diag", (128, DT * D_CONV * 128), F16)  # per d-tile, 4 diag mats
    di("conv_b", (128, DT), F32)
    di("w_x", (128, DT, 33), F16)          # W_x shard^T k-tiles
    di("w_dt", (128, DT), F32)
    di("b_dt", (128, DT), F32)
    di("a_mat", (128, DT, D_STATE), F32)   # A = -exp(A_log) shard
    di("d_vec", (128, DT), F32)
    di("w_out", (128, DT, D_MODEL), F16)   # W_out shard^T k-tiles
    di("ones_row", (1, 128), F16)
    di("ident", (128, 128), F16)

    bc_scratch = nc.dram_tensor("bc_scratch", [2 * D_STATE, L], F16).ap()
    out_d = nc.dram_tensor("out_partial", [D_MODEL, L], F32,
                           kind="ExternalOutput").ap()

    with tile.TileContext(nc) as tc:
        with ExitStack() as ctx:
            consts = ctx.enter_context(tc.tile_pool(name="consts", bufs=1))
            big = ctx.enter_context(tc.tile_pool(name="big", bufs=1))
            ph1 = ctx.enter_context(tc.tile_pool(name="ph1", bufs=1))
            psum = ctx.enter_context(tc.tile_pool(name="psum", bufs=2, space="PSUM"))
            psum_y = ctx.enter_context(tc.tile_pool(name="psum_y", bufs=1, space="PSUM"))
            scanp = ctx.enter_context(tc.tile_pool(name="scanp", bufs=3))
            bcp = ctx.enter_context(tc.tile_pool(name="bcp", bufs=2))
            outp = ctx.enter_context(tc.tile_pool(name="outp", bufs=3))

            def load(name, pool=consts):
                t = pool.tile(list(d[name].shape), d[name].dtype, tag=name, name=name)
                nc.sync.dma_start(t[:], d[name][:])
                return t

            hT = load("hT", ph1)
            w_in = load("w_in")
            conv_diag = load("conv_diag")
            conv_b = load("conv_b")
            w_x = load("w_x")
            w_dt = load("w_dt")
            b_dt = load("b_dt")
            a_mat = load("a_mat")
            d_vec = load("d_vec")
            w_out = load("w_out")
            ones_row = load("ones_row")
            ident = load("ident")

            # ---- Phase 1: in_proj (x and z), conv, silu, x_dbl, delta, g ----
            x_pre = [ph1.tile([128, L + D_CONV - 1], F16, tag=f"x_pre{i}", name=f"x_pre{i}")
                     for i in range(DT)]
            x = [big.tile([128, L], F16, tag=f"x{i}", name=f"x{i}") for i in range(DT)]
            sz = [big.tile([128, L], F16, tag=f"sz{i}", name=f"sz{i}") for i in range(DT)]
            delta = [big.tile([128, L], F16, tag=f"delta{i}", name=f"delta{i}") for i in range(DT)]
            g = [big.tile([128, L], F16, tag=f"g{i}", name=f"g{i}") for i in range(DT)]
            y3 = [big.tile([128, L], F16, tag=f"y3{i}", name=f"y3{i}") for i in range(DT)]
            carry = [big.tile([128, D_STATE], F32, tag=f"carry{i}", name=f"carry{i}")
                     for i in range(DT)]

            for i in range(DT):
                nc.vector.memset(x_pre[i][:, 0:D_CONV - 1], 0.0)

            # in_proj: out rows 0..D_LOC-1 = x, D_LOC..2*D_LOC-1 = z
            for mi in range(2 * DT):
                for c in range(NCH):
                    ps = psum.tile([128, 512], F32, tag="mm")
                    for k in range(KT):
                        nc.tensor.matmul(
                            ps[:],
                            w_in[:, k, mi * 128:(mi + 1) * 128],
                            hT[:, k, c * 512:(c + 1) * 512],
                            start=(k == 0), stop=(k == KT - 1))
                    if mi < DT:
                        nc.scalar.copy(
                            x_pre[mi][:, D_CONV - 1 + c * 512:
                                      D_CONV - 1 + (c + 1) * 512], ps[:])
                    else:
                        nc.scalar.activation(
                            sz[mi - DT][:, c * 512:(c + 1) * 512], ps[:],
                            AF.Silu)

            # depthwise causal conv via diagonal matmuls + SiLU(+bias)
            for i in range(DT):
                for c in range(NCH):
                    ps = psum.tile([128, 512], F32, tag="mm")
                    for k in range(D_CONV):
                        nc.tensor.matmul(
                            ps[:],
                            conv_diag[:, (i * D_CONV + k) * 128:
                                      (i * D_CONV + k + 1) * 128],
                            x_pre[i][:, c * 512 + k:c * 512 + k + 512],
                            start=(k == 0), stop=(k == D_CONV - 1))
                    nc.scalar.activation(
                        x[i][:, c * 512:(c + 1) * 512], ps[:],
                        AF.Silu, bias=conv_b[:, i:i + 1])

            # x_dbl = W_x @ x : (33, L) psum; keep fp16 copy
            xdbl = ph1.tile([33, L], F16, tag="xdbl")
            for c in range(NCH):
                ps = psum.tile([33, 512], F32, tag="mm")
                for i in range(DT):
                    nc.tensor.matmul(ps[:], w_x[:, i, :],
                                     x[i][:, c * 512:(c + 1) * 512],
                                     start=(i == 0), stop=(i == DT - 1))
                nc.scalar.copy(xdbl[:, c * 512:(c + 1) * 512], ps[:])

            # bounce B/C rows to DRAM for partition-broadcast reads
            nc.sync.dma_start(bc_scratch[:], xdbl[1:33, :])

            # broadcast dt-scalar row to 128 partitions via ones matmul
            s_bc = ph1.tile([128, L], F32, tag="s_bc")
            for c in range(NCH):
                ps = psum.tile([128, 512], F32, tag="mm")
                nc.tensor.matmul(ps[:], ones_row[:],
                                 xdbl[0:1, c * 512:(c + 1) * 512],
                                 start=True, stop=True)
                nc.scalar.copy(s_bc[:, c * 512:(c + 1) * 512], ps[:])

            # delta = softplus(w_dt * s + b_dt) = Ln(Exp(...) + 1)
            for i in range(DT):
                e_tmp = ph1.tile([128, L], F32, tag="e_tmp")
                nc.scalar.activation(e_tmp[:], s_bc[:], AF.Exp,
                                     scale=w_dt[:, i:i + 1],
                                     bias=b_dt[:, i:i + 1])
                nc.scalar.activation(delta[i][:], e_tmp[:], AF.Ln, bias=1.0)
                nc.vector.tensor_mul(g[i][:], delta[i][:], x[i][:])

            # ---- Phase 2: selective scan over two l-halves ----
            for half in range(2):
                lo = half * H
                ys = [psum_y.tile([128, H], F32, tag=f"ys{i}", name=f"ys{i}") for i in range(DT)]
                for n in range(D_STATE):
                    bb = bcp.tile([128, H], F16, tag="bb")
                    nc.sync.dma_start(
                        bb[:], bc_scratch[n:n + 1, lo:lo + H].broadcast_to((128, H)))
                    cb = bcp.tile([128, H], F16, tag="cb")
                    nc.sync.dma_start(
                        cb[:], bc_scratch[D_STATE + n:D_STATE + n + 1,
                                          lo:lo + H].broadcast_to((128, H)))
                    for i in range(DT):
                        dA = scanp.tile([128, H], F16, tag="dA")
                        nc.scalar.activation(dA[:], delta[i][:, lo:lo + H],
                                             AF.Exp, scale=a_mat[:, i, n:n + 1])
                        dBu = scanp.tile([128, H], F16, tag="dBu")
                        nc.vector.tensor_mul(dBu[:], g[i][:, lo:lo + H], bb[:])
                        h = scanp.tile([128, H], F16, tag="h")
                        init = 0.0 if half == 0 else carry[i][:, n:n + 1]
                        nc.vector.tensor_tensor_scan(h[:], dA[:], dBu[:], init,
                                                     OP.mult, OP.add)
                        if half == 0:
                            nc.vector.tensor_copy(carry[i][:, n:n + 1],
                                                  h[:, H - 1:H])
                        hc = scanp.tile([128, H], F16, tag="hc")
                        nc.vector.tensor_mul(hc[:], h[:], cb[:])
                        for c in range(HCH):
                            nc.tensor.matmul(
                                ys[i][:, c * 512:(c + 1) * 512], ident[:],
                                hc[:, c * 512:(c + 1) * 512],
                                start=(n == 0), stop=(n == D_STATE - 1))

                # gating: y3 = (x * D + y) * silu(z)
                for i in range(DT):
                    y_sb = outp.tile([128, H], F16, tag="y_sb")
                    nc.scalar.copy(y_sb[:], ys[i][:])
                    y2 = outp.tile([128, H], F16, tag="y2")
                    nc.vector.scalar_tensor_tensor(
                        y2[:], x[i][:, lo:lo + H], d_vec[:, i:i + 1], y_sb[:],
                        OP.mult, OP.add)
                    nc.vector.tensor_mul(y3[i][:, lo:lo + H], y2[:], sz[i][:, lo:lo + H])

                # out_proj for this half
                for mi in range(MT):
                    for c in range(HCH):
                        ps = psum.tile([128, 512], F32, tag="mm")
                        for i in range(DT):
                            nc.tensor.matmul(
                                ps[:], w_out[:, i, mi * 128:(mi + 1) * 128],
                                y3[i][:, lo + c * 512:lo + (c + 1) * 512],
                                start=(i == 0), stop=(i == DT - 1))
                        ostage = outp.tile([128, 512], F32, tag="ostage")
                        nc.scalar.copy(ostage[:], ps[:])
                        nc.sync.dma_start(
                            out_d[mi * 128:(mi + 1) * 128,
                                  lo + c * 512:lo + (c + 1) * 512], ostage[:])

    nc.compile()
    return nc


def _shard_inputs(inputs):
    """Build the 8 per-core input dicts (host-side layout/dtype prep)."""
    hs = np.asarray(inputs["hidden_states"], np.float32)
    W_in = np.asarray(inputs["W_in"], np.float32)
    conv_w = np.asarray(inputs["conv_w"], np.float32)
    conv_b = np.asarray(inputs["conv_b"], np.float32)
    W_x = np.asarray(inputs["W_x"], np.float32)
    W_dt = np.asarray(inputs["W_dt"], np.float32)
    b_dt = np.asarray(inputs["b_dt"], np.float32)
    A_log = np.asarray(inputs["A_log"], np.float32)
    D = np.asarray(inputs["D"], np.float32)
    W_out = np.asarray(inputs["W_out"], np.float32)

    A = -np.exp(A_log)                                   # (D_INNER, 16)
    ktile = lambda a: np.ascontiguousarray(              # (rows, cols) ->
        a.reshape(-1, 128, a.shape[-1]).transpose(1, 0, 2))  # (128, kt, cols)

    in_maps = []
    for core in range(N_CORES):
        b, s = divmod(core, D_SHARDS)
        d0 = s * D_LOC
        sl = slice(d0, d0 + D_LOC)
        zl = slice(D_INNER + d0, D_INNER + d0 + D_LOC)

        hT = hs[b].T                                     # (768, L)
        w_in_x = W_in[sl].T                              # (768, D_LOC)
        w_in_z = W_in[zl].T
        w_in = np.concatenate([w_in_x, w_in_z], 1)       # (768, 2*D_LOC)

        cw = conv_w[sl, 0, :]                            # (D_LOC, 4)
        diags = np.zeros((128, DT * D_CONV * 128), np.float16)
        for i in range(DT):
            for k in range(D_CONV):
                blk = (i * D_CONV + k) * 128
                np.fill_diagonal(diags[:, blk:blk + 128],
                                 cw[i * 128:(i + 1) * 128, k].astype(np.float16))

        pcol = lambda v: np.ascontiguousarray(
            v.reshape(DT, 128).T.astype(np.float32))     # (128, DT)

        m = {
            "hT": ktile(hT).astype(np.float16),
            "w_in": ktile(w_in).astype(np.float16),
            "conv_diag": diags,
            "conv_b": pcol(conv_b[sl]),
            "w_x": ktile(W_x[:, sl].T).astype(np.float16),
            "w_dt": pcol(W_dt[sl, 0]),
            "b_dt": pcol(b_dt[sl]),
            "a_mat": np.ascontiguousarray(
                A[sl].reshape(DT, 128, D_STATE).transpose(1, 0, 2)).astype(np.float32),
            "d_vec": pcol(D[sl]),
            "w_out": ktile(W_out[:, sl].T).astype(np.float16),
            "ones_row": np.ones((1, 128), np.float16),
            "ident": np.eye(128, dtype=np.float16),
        }
        in_maps.append(m)
    return in_maps


def kernel(**inputs):
    from concourse.bass_utils import run_bass_kernel_spmd

    if "prog" not in _PROG_CACHE:
        _PROG_CACHE["prog"] = _build_program()
    nc = _PROG_CACHE["prog"]

    in_maps = _shard_inputs(inputs)
    res = run_bass_kernel_spmd(nc, in_maps, core_ids=list(range(N_CORES)),
                               **_PROG_CACHE.get("run_kwargs", {}))
    _PROG_CACHE["last_result"] = res

    out = np.zeros((BATCH, L, D_MODEL), np.float32)
    for b in range(BATCH):
        acc = np.zeros((D_MODEL, L), np.float32)
        for s in range(D_SHARDS):
            acc += res.results[b * D_SHARDS + s]["out_partial"]
        out[b] = acc.T
    return out
